# revision 13
# baseline (speedup 1.0000x reference)
"""AtlasSpecializedLoss on 8 TRN2 NeuronCores — pure data parallel over B.

Each core processes B/8 = 512 images. The device computes per-image partial
reductions (match counts, focal/edge sums, softmax channel sums, per-color
row/col marginals); the host finishes the tiny O(B) scalar assembly (incl.
the 36-pair geo compaction) in float64. No collective needed — per-core
outputs are gathered host-side.

Device layout: partition = image (128 per tile, 4 tiles/core), free axis =
[C=10, 900 pixels] (36 KB/partition). target/input_grid stream in 5-channel
chunks. All transcendentals use only Exp/Ln/Copy/Square (one ACT table set;
sqrt is computed as exp(0.5*ln(x)) to avoid table switches).
"""

import sys

for _p in ("/opt/trn_rl_repo", "/opt/pypackages"):
    if _p not in sys.path:
        sys.path.append(_p)

import numpy as np

import concourse.bass as bass
import concourse.bacc as bacc
from concourse import mybir
from concourse.tile import TileContext
from concourse.bass_utils import run_bass_kernel_spmd

F32 = mybir.dt.float32
AF = mybir.ActivationFunctionType
OP = mybir.AluOpType
AX = mybir.AxisListType

B, C, H, W = 4096, 10, 30, 30
PIX = H * W
NCORE = 8
BS = B // NCORE            # 512 images per core
P = 128                    # partitions = images per tile
NT = BS // P               # 4 tiles
CH = 5                     # channel-chunk size for target/input streaming

# per-image output layout (floats)
O_MPT, O_MCP, O_FOC, O_EDG, O_AFF, O_ROT, O_RFL = 0, 1, 2, 3, 4, 5, 6
O_PC = 10                  # 10: softmax channel sums
O_RP = 20                  # 9*30: pred rowcnt, colors 1..9
O_CP = 290                 # 9*30: pred colcnt
O_RT = 560                 # 10*30: target rowcnt, colors 0..9
O_CT = 860                 # 9*30: target colcnt, colors 1..9
OUTW = 1152


def _hw(ap):
    """[P, 900] -> [P, 30h, 30w] view."""
    return ap.rearrange("p (h w) -> p h w", w=W)


def _wh(ap):
    """[P, 900] -> [P, 30w, 30h] view (transposed access)."""
    return ap.rearrange("p (h w) -> p w h", w=W)


def build_graph(phases: str = "12345erp") -> bass.Bass:
    nc = bacc.Bacc()
    pred = nc.declare_dram_parameter("pred", [BS, C * PIX], F32, isOutput=False)
    targ = nc.declare_dram_parameter("targ", [BS, C * PIX], F32, isOutput=False)
    ig = nc.declare_dram_parameter("ig", [BS, C * PIX], F32, isOutput=False)
    theta = nc.declare_dram_parameter("theta", [BS, 6], F32, isOutput=False)
    rot = nc.declare_dram_parameter("rot", [BS, 8], F32, isOutput=False)
    refl = nc.declare_dram_parameter("refl", [BS, 4], F32, isOutput=False)
    out = nc.declare_dram_parameter("out", [BS, OUTW], F32, isOutput=True)

    v = nc.vector
    a = nc.scalar

    with TileContext(nc) as tc:
        with (
            tc.tile_pool(name="pr", bufs=1) as prp,
            tc.tile_pool(name="tg", bufs=2) as tgp,
            tc.tile_pool(name="igc", bufs=2) as igp,
            tc.tile_pool(name="per", bufs=1) as per,
            tc.tile_pool(name="scr", bufs=4) as scr,
            tc.tile_pool(name="outp", bufs=2) as outp,
            tc.tile_pool(name="tiny", bufs=8) as tiny,
        ):
            for t in range(NT):
                r0 = t * P

                pr = prp.tile([P, C, PIX], F32, tag="pr")
                nc.sync.dma_start(out=pr, in_=pred[r0:r0 + P, :].rearrange("p (c x) -> p c x", c=C))

                ot = outp.tile([P, OUTW], F32, tag="ot")
                nc.gpsimd.memset(ot, 0.0)

                # ---- phase 1: consume target (streamed in 5-ch chunks) ----
                tidx = per.tile([P, PIX], F32, tag="tidx")
                ptacc = per.tile([P, PIX], F32, tag="ptacc")
                for g in range(C // CH) if "1" in phases else []:
                    c0 = g * CH
                    tg_t = tgp.tile([P, CH, PIX], F32, tag="tg")
                    nc.sync.dma_start(
                        out=tg_t,
                        in_=targ[r0:r0 + P, c0 * PIX:(c0 + CH) * PIX].rearrange(
                            "p (c x) -> p c x", c=CH),
                    )
                    for cc in range(CH):
                        c = c0 + cc
                        tgc = tg_t[:, cc, :]
                        # tidx accumulation: sum_c c*onehot
                        if c == 1:
                            a.activation(tidx, tgc, AF.Copy, bias=0.0, scale=1.0)
                        elif c >= 2:
                            q = scr.tile([P, PIX], F32, tag="scr")
                            a.activation(q, tgc, AF.Copy, bias=0.0, scale=float(c))
                            v.tensor_add(tidx, tidx, q)
                        # target row/col marginals
                        v.tensor_reduce(ot[:, O_RT + 30 * c:O_RT + 30 * (c + 1)],
                                        _hw(tgc), axis=AX.X, op=OP.add)
                        if c >= 1:
                            v.tensor_reduce(ot[:, O_CT + 30 * (c - 1):O_CT + 30 * c],
                                            _wh(tgc), axis=AX.X, op=OP.add)
                        # pred logit at target class: sum_c onehot*pred
                        if c == 0:
                            v.tensor_mul(ptacc, tgc, pr[:, 0, :])
                        else:
                            q2 = scr.tile([P, PIX], F32, tag="scr")
                            v.tensor_mul(q2, tgc, pr[:, c, :])
                            v.tensor_add(ptacc, ptacc, q2)

                # ---- phase 2: pred-only reductions ----
                if "2" not in phases:
                    nc.sync.dma_start(out=out[r0:r0 + P, :], in_=ot)
                    continue
                mx = per.tile([P, PIX], F32, tag="mx")
                if True:
                    pass
                v.tensor_max(mx, pr[:, 0, :], pr[:, 1, :])
                for c in range(2, C):
                    v.tensor_max(mx, mx, pr[:, c, :])
                # first-argmax: macc = max_c eq_c*(1000-c); pidx = 1000-macc
                macc = per.tile([P, PIX], F32, tag="macc")
                for c in range(C):
                    eq = scr.tile([P, PIX], F32, tag="scr")
                    v.tensor_tensor(eq, pr[:, c, :], mx, OP.is_equal)
                    if c == 0:
                        a.activation(macc, eq, AF.Copy, bias=0.0, scale=1000.0)
                    else:
                        q = scr.tile([P, PIX], F32, tag="scr")
                        a.activation(q, eq, AF.Copy, bias=0.0, scale=float(1000 - c))
                        v.tensor_max(macc, macc, q)
                pidx = per.tile([P, PIX], F32, tag="pidx")
                a.activation(pidx, macc, AF.Copy, bias=1000.0, scale=-1.0)
                # softmax denominator (no max shift needed: |pred| small)
                sacc = per.tile([P, PIX], F32, tag="sacc")
                if "e" not in phases:
                    nc.sync.dma_start(out=out[r0:r0 + P, :], in_=ot)
                    continue
                a.activation(sacc, pr[:, 0, :], AF.Exp)
                for c in range(1, C):
                    e = scr.tile([P, PIX], F32, tag="scr")
                    a.activation(e, pr[:, c, :], AF.Exp)
                    v.tensor_add(sacc, sacc, e)
                rr = per.tile([P, PIX], F32, tag="rr")
                if "r" not in phases:
                    nc.sync.dma_start(out=out[r0:r0 + P, :], in_=ot)
                    continue
                v.reciprocal(rr, sacc)
                # pc[c] = sum_pix softmax
                if "p" not in phases:
                    nc.sync.dma_start(out=out[r0:r0 + P, :], in_=ot)
                    continue
                for c in range(C):
                    e = scr.tile([P, PIX], F32, tag="scr")
                    a.activation(e, pr[:, c, :], AF.Exp)
                    v.tensor_mul(e, e, rr)
                    a.activation(e, e, AF.Copy, bias=0.0, scale=1.0,
                                 accum_out=ot[:, O_PC + c:O_PC + c + 1])

                # ---- phase 3: pidx-based counts/moments + input_grid ----
                if "3" not in phases:
                    nc.sync.dma_start(out=out[r0:r0 + P, :], in_=ot)
                    continue
                eqv = scr.tile([P, PIX], F32, tag="scr")
                v.tensor_tensor(eqv, pidx, tidx, OP.is_equal)
                a.activation(eqv, eqv, AF.Copy, bias=0.0, scale=1.0,
                             accum_out=ot[:, O_MPT:O_MPT + 1])
                cm = tiny.tile([P, C], F32, tag="cm")
                for g in range(C // CH):
                    c0 = g * CH
                    ig_t = igp.tile([P, CH, PIX], F32, tag="igc")
                    nc.sync.dma_start(
                        out=ig_t,
                        in_=ig[r0:r0 + P, c0 * PIX:(c0 + CH) * PIX].rearrange(
                            "p (c x) -> p c x", c=CH),
                    )
                    for cc in range(CH):
                        c = c0 + cc
                        mc = scr.tile([P, PIX], F32, tag="scr")
                        v.tensor_scalar(mc, pidx, float(c), None, OP.is_equal)
                        prod = scr.tile([P, PIX], F32, tag="scr")
                        v.tensor_mul(prod, mc, ig_t[:, cc, :])
                        a.activation(prod, prod, AF.Copy, bias=0.0, scale=1.0,
                                     accum_out=cm[:, c:c + 1])
                        if c >= 1:
                            v.tensor_reduce(ot[:, O_RP + 30 * (c - 1):O_RP + 30 * c],
                                            _hw(mc), axis=AX.X, op=OP.add)
                            v.tensor_reduce(ot[:, O_CP + 30 * (c - 1):O_CP + 30 * c],
                                            _wh(mc), axis=AX.X, op=OP.add)
                v.tensor_reduce(ot[:, O_MCP:O_MCP + 1], cm, axis=AX.X, op=OP.add)

                # ---- phase 4a: edge weights from tidx ----
                if "4" not in phases:
                    nc.sync.dma_start(out=out[r0:r0 + P, :], in_=ot)
                    continue
                ee = scr.tile([P, PIX], F32, tag="scr")
                v.memset(ee, 0.0)
                dh = scr.tile([P, PIX], F32, tag="scr")
                v.tensor_tensor(dh[:, :870], tidx[:, 30:], tidx[:, :870], OP.not_equal)
                v.tensor_add(ee[:, 30:], ee[:, 30:], dh[:, :870])
                v.tensor_add(ee[:, :870], ee[:, :870], dh[:, :870])
                dw = scr.tile([P, PIX], F32, tag="scr")
                dwv = dw[:, :870].rearrange("p (h w) -> p h w", w=29)
                v.tensor_tensor(dwv, _hw(tidx)[:, :, 1:], _hw(tidx)[:, :, :29],
                                OP.not_equal)
                v.tensor_add(_hw(ee)[:, :, 1:], _hw(ee)[:, :, 1:], dwv)
                v.tensor_add(_hw(ee)[:, :, :29], _hw(ee)[:, :, :29], dwv)
                sw = per.tile([P, PIX], F32, tag="sw")
                v.tensor_scalar(sw, ee, 0.0, None, OP.is_gt)
                a.activation(sw, sw, AF.Copy, bias=1.0, scale=0.5)

                # ---- phase 4b: focal loss ----
                ce = scr.tile([P, PIX], F32, tag="scr")
                a.activation(ce, sacc, AF.Ln)
                v.tensor_sub(ce, ce, ptacc)          # ce = ln(s) - pred_t
                pt = scr.tile([P, PIX], F32, tag="scr")
                a.activation(pt, ptacc, AF.Exp)
                v.tensor_mul(pt, pt, rr)             # pt = exp(pred_t)/s
                a.activation(pt, pt, AF.Copy, bias=1.0, scale=-1.0)   # u = 1-pt
                v.tensor_scalar_max(pt, pt, 1e-30)
                a.activation(pt, pt, AF.Ln)
                a.activation(pt, pt, AF.Exp, scale=1.4)  # (1-pt)^1.4
                v.tensor_mul(ce, ce, pt)
                v.tensor_mul(ce, ce, sw)
                a.activation(ce, ce, AF.Copy, bias=0.0, scale=1.0,
                             accum_out=ot[:, O_FOC:O_FOC + 1])

                # ---- phase 4c: sobel edge MSE on pidx/tidx ----
                def sobel(idxf, dst):
                    S = scr.tile([P, PIX], F32, tag="scr")
                    a.activation(S, idxf, AF.Copy, bias=0.0, scale=2.0)
                    v.tensor_add(S[:, 30:], S[:, 30:], idxf[:, :870])
                    v.tensor_add(S[:, :870], S[:, :870], idxf[:, 30:])
                    EX = scr.tile([P, PIX], F32, tag="scr")
                    vS, vE = _hw(S), _hw(EX)
                    a.activation(vE[:, :, 0:1], vS[:, :, 1:2], AF.Copy, bias=0.0, scale=1.0)
                    a.activation(vE[:, :, 29:30], vS[:, :, 28:29], AF.Copy, bias=0.0, scale=-1.0)
                    v.tensor_sub(vE[:, :, 1:29], vS[:, :, 2:], vS[:, :, :28])
                    T = scr.tile([P, PIX], F32, tag="scr")
                    a.activation(T, idxf, AF.Copy, bias=0.0, scale=2.0)
                    vI, vT = _hw(idxf), _hw(T)
                    v.tensor_add(vT[:, :, 1:], vT[:, :, 1:], vI[:, :, :29])
                    v.tensor_add(vT[:, :, :29], vT[:, :, :29], vI[:, :, 1:])
                    EY = scr.tile([P, PIX], F32, tag="scr")
                    a.activation(EY[:, :30], T[:, 30:60], AF.Copy, bias=0.0, scale=1.0)
                    a.activation(EY[:, 870:], T[:, 840:870], AF.Copy, bias=0.0, scale=-1.0)
                    v.tensor_sub(EY[:, 30:870], T[:, 60:], T[:, :840])
                    a.square(EX, EX)
                    a.square(EY, EY)
                    v.tensor_add(EX, EX, EY)
                    # sqrt via exp(0.5*ln) to stay in one ACT table set
                    v.tensor_scalar_max(EX, EX, 1e-30)
                    a.activation(EX, EX, AF.Ln)
                    a.activation(dst, EX, AF.Exp, scale=0.5)

                pe = per.tile([P, PIX], F32, tag="pe")
                te = per.tile([P, PIX], F32, tag="te")
                sobel(pidx, pe)
                sobel(tidx, te)
                v.tensor_sub(pe, pe, te)
                a.activation(pe, pe, AF.Square, accum_out=ot[:, O_EDG:O_EDG + 1])

                # ---- phase 4d: theta / rotation / reflection ----
                if "5" not in phases:
                    nc.sync.dma_start(out=out[r0:r0 + P, :], in_=ot)
                    continue
                th = tiny.tile([P, 6], F32, tag="th")
                nc.sync.dma_start(out=th, in_=theta[r0:r0 + P, :])
                a.square(th, th)
                ssum = tiny.tile([P, 2], F32, tag="ssum")
                v.tensor_reduce(ssum[:, 0:1],
                                th.rearrange("p (r k) -> p r k", k=3)[:, :, 0:2],
                                axis=AX.XY, op=OP.add)
                v.tensor_reduce(ssum[:, 1:2],
                                th.rearrange("p (r k) -> p r k", k=3)[:, :, 2:3],
                                axis=AX.XY, op=OP.add)
                v.tensor_scalar_max(ssum, ssum, 1e-30)
                a.activation(ssum, ssum, AF.Ln)
                a.activation(ssum, ssum, AF.Exp, scale=0.5)   # sqrt
                q = tiny.tile([P, 1], F32, tag="q1")
                a.activation(q, ssum[:, 1:2], AF.Copy, bias=0.0, scale=0.1)
                v.tensor_add(ot[:, O_AFF:O_AFF + 1], ssum[:, 0:1], q)

                def entropy(src, n, dst, tagp):
                    lg = tiny.tile([P, n], F32, tag=tagp)
                    nc.sync.dma_start(out=lg, in_=src[r0:r0 + P, :])
                    m8 = tiny.tile([P, 1], F32, tag=tagp + "m")
                    v.tensor_reduce(m8, lg, axis=AX.X, op=OP.max)
                    nm = tiny.tile([P, 1], F32, tag=tagp + "n")
                    a.activation(nm, m8, AF.Copy, bias=0.0, scale=-1.0)
                    z8 = tiny.tile([P, n], F32, tag=tagp + "z")
                    v.tensor_scalar(z8, lg, nm, None, OP.add)
                    e8 = tiny.tile([P, n], F32, tag=tagp + "e")
                    a.activation(e8, lg, AF.Exp, bias=nm)
                    s8 = tiny.tile([P, 1], F32, tag=tagp + "s")
                    v.tensor_reduce(s8, e8, axis=AX.X, op=OP.add)
                    dot = tiny.tile([P, 1], F32, tag=tagp + "d")
                    dsk = tiny.tile([P, n], F32, tag=tagp + "k")
                    v.tensor_mul(dsk, e8, z8)
                    v.tensor_reduce(dot, dsk, axis=AX.X, op=OP.add)
                    r8 = tiny.tile([P, 1], F32, tag=tagp + "r")
                    v.reciprocal(r8, s8)
                    v.tensor_mul(dot, dot, r8)
                    a.activation(s8, s8, AF.Ln)
                    v.tensor_sub(dst, s8, dot)

                entropy(rot, 8, ot[:, O_ROT:O_ROT + 1], "ro")
                entropy(refl, 4, ot[:, O_RFL:O_RFL + 1], "rf")

                nc.sync.dma_start(out=out[r0:r0 + P, :], in_=ot)
    nc.finalize()
    return nc


_GRAPH = None


def _get_graph():
    global _GRAPH
    if _GRAPH is None:
        _GRAPH = build_graph()
    return _GRAPH


def run_device(inputs: dict, trace: bool = False):
    pred = np.asarray(inputs["pred_output"], np.float32).reshape(B, C * PIX)
    targ = np.asarray(inputs["target_output"], np.float32).reshape(B, C * PIX)
    igrid = np.asarray(inputs["input_grid"], np.float32).reshape(B, C * PIX)
    theta = np.asarray(inputs["theta"], np.float32).reshape(B, 6)
    rot = np.asarray(inputs["rotation_logits"], np.float32).reshape(B, 8)
    refl = np.asarray(inputs["reflection_logits"], np.float32).reshape(B, 4)

    in_maps = []
    for i in range(NCORE):
        s = slice(i * BS, (i + 1) * BS)
        in_maps.append({
            "pred": np.ascontiguousarray(pred[s]),
            "targ": np.ascontiguousarray(targ[s]),
            "ig": np.ascontiguousarray(igrid[s]),
            "theta": np.ascontiguousarray(theta[s]),
            "rot": np.ascontiguousarray(rot[s]),
            "refl": np.ascontiguousarray(refl[s]),
        })
    res = run_bass_kernel_spmd(_get_graph(), in_maps, core_ids=list(range(NCORE)),
                               trace=trace)
    outs = np.concatenate([r["out"] for r in res.results], axis=0)  # [B, OUTW]
    return outs, res


def assemble(outs: np.ndarray) -> np.ndarray:
    o = outs.astype(np.float64)
    npix = float(B * PIX)
    match_pt, match_cp = o[:, O_MPT], o[:, O_MCP]
    spatial_focal = o[:, O_FOC].sum() / npix
    exact = match_pt == PIX
    exact_count = exact.sum()
    exact_bonus = -exact.mean() * 7.0
    transform = (match_cp == PIX).mean() * 0.2
    affine = o[:, O_AFF].mean() * 0.4
    rotation = o[:, O_ROT].mean() * 0.3
    reflection = o[:, O_RFL].mean() * 0.3
    edge = o[:, O_EDG].sum() / npix * 0.3

    pc = o[:, O_PC:O_PC + 10]
    rows_p = o[:, O_RP:O_RP + 270].reshape(B, 9, 30)
    cols_p = o[:, O_CP:O_CP + 270].reshape(B, 9, 30)
    rows_t = o[:, O_RT:O_RT + 300].reshape(B, 10, 30)
    cols_t = o[:, O_CT:O_CT + 270].reshape(B, 9, 30)

    tc_full = rows_t.sum(2)
    pcn = pc / (pc.sum(1, keepdims=True) + 1e-8)
    tcn = tc_full / (tc_full.sum(1, keepdims=True) + 1e-8)
    cbal = ((pcn - tcn) ** 2).mean() * 0.2

    hh = np.arange(H, dtype=np.float64)
    ww = np.arange(W, dtype=np.float64)

    def centers(rows, cols):
        cnt = rows.sum(2)
        cy = (rows * hh).sum(2) / np.maximum(cnt, 1.0)
        cx = (cols * ww).sum(2) / np.maximum(cnt, 1.0)
        return cy, cx, cnt > 0

    cyp, cxp, prp = centers(rows_p, cols_p)
    cyt, cxt, prt = centers(rows_t[:, 1:, :], cols_t)
    PI, PJ = np.triu_indices(9, 1)
    NP = PI.shape[0]

    def compact(cy, cx, pres):
        d = np.sqrt((cy[:, PI] - cy[:, PJ]) ** 2 + (cx[:, PI] - cx[:, PJ]) ** 2)
        vv = pres[:, PI] & pres[:, PJ]
        rank = np.cumsum(vv, axis=1) - 1
        slot = np.where(vv, rank, NP)
        comp = np.zeros((B, NP + 1))
        np.put_along_axis(comp, slot, d, axis=1)
        return comp[:, :NP], vv.sum(1)

    dpc, n_p = compact(cyp, cxp, prp)
    dtc, n_t = compact(cyt, cxt, prt)
    m = np.minimum(n_p, n_t)
    use = np.arange(NP)[None, :] < m[:, None]
    sq = (((dpc - dtc) ** 2) * use).sum(1)
    geo_b = np.where(m > 0, sq / np.maximum(m, 1), 0.0)
    geo = geo_b.sum() / B * 0.5

    total = (spatial_focal + transform + affine + rotation + reflection
             + geo + edge + cbal + exact_bonus)
    return np.array([total, spatial_focal, transform, exact_bonus, exact_count,
                     affine, rotation, reflection, geo, edge, cbal], np.float32)


def kernel(**inputs) -> np.ndarray:
    outs, _ = run_device(inputs, trace=False)
    return assemble(outs)
